# revision 18
# baseline (speedup 1.0000x reference)
"""Distributed Trainium2 Bass kernel for the GNN message-passing network
(node/edge LSTMCells + MPN linears, 20 iterations).

Strategy (8 NeuronCores):
  - Nodes sharded contiguously: core r owns nodes [r*1250, (r+1)*1250), padded
    to 1280 local slots. Edges sharded by dst owner and host-sorted by dst, so
    the scatter-mean is core-local.
  - On-chip layouts are feature-major (features on partitions, tokens on the
    free dim), which feeds the TensorEngine's K-on-partitions contraction
    directly for every matmul of the model.
  - Per iteration the (bf16) node states are AllGathered through a DRAM table;
    h_node[src] / h_node[dst] are fetched with transposing dma_gathers. The
    scatter-mean is computed with prebuilt 0/1 aggregation matrices on the
    TensorEngine (dst is sorted + local, so these are narrow banded blocks).
  - LSTM cell state c stays fp32 on-chip; h states are bf16 (matmul inputs).
    PSUM accumulation is fp32.
"""

import numpy as np
import ml_dtypes

import concourse.bacc as bacc
import concourse.mybir as mybir
from concourse.tile import TileContext
from concourse.bass_utils import run_bass_kernel_spmd

BF16 = ml_dtypes.bfloat16

H = 128
import os as _os
NUM_ITER = int(_os.environ.get("KITER", "20"))
SKIP = set(filter(None, _os.environ.get("KSKIP", "").split(",")))
NF = 16          # node feats
EF = 20          # edge feats
NCLS = 4
NEG = 0.01       # LeakyReLU slope
NCORES = 8
N = 10000
NL = N // NCORES        # 1250 real nodes per core
NLP = 1280              # padded local nodes (10 chunks of 128)
NCH = NLP // 128        # node chunks
SUP = 1024              # gate supertile (tokens)
GCH = 2048              # dma_gather chunk (indices per call)

TRACE = False           # set by test harness for profiling
LAST_RESULT = None      # BassKernelResults stash for the harness

AF = mybir.ActivationFunctionType


def _roundup(a, b):
    return (a + b - 1) // b * b


def _wrap_idx(idx):
    """dma_gather index layout: idx i at [i%16, i//16], replicated 8x on the
    partition axis."""
    n = len(idx)
    arr = idx.reshape(n // 16, 16).T.astype(np.int16)
    return np.tile(arr, (8, 1))


def _preprocess(inputs):
    x = np.asarray(inputs["x"], np.float32)
    edge_index = np.asarray(inputs["edge_index"]).astype(np.int64)
    edge_attr = np.asarray(inputs["edge_attr"], np.float32)
    W_fc1 = np.asarray(inputs["W_fc1"], np.float32)
    b_fc1 = np.asarray(inputs["b_fc1"], np.float32)

    src, dst = edge_index[0], edge_index[1]
    order = np.argsort(dst, kind="stable")
    src_s, dst_s = src[order], dst[order]
    ea_s = edge_attr[order]
    bounds = np.searchsorted(dst_s, np.arange(NCORES + 1) * NL)
    E_r = np.diff(bounds)
    EP = _roundup(int(E_r.max()), 2048)

    # host edge_feat (iteration-0 edge LSTM input)
    edge_feat = ea_s @ W_fc1.T + b_fc1  # [E, EF] f32

    counts_full = np.bincount(dst_s, minlength=N).astype(np.float32)
    counts_full = np.maximum(counts_full, 1.0)

    cores = []
    for r in range(NCORES):
        lo, hi = int(bounds[r]), int(bounds[r + 1])
        e = hi - lo
        srcr = src_s[lo:hi]
        dstl = (dst_s[lo:hi] - r * NL).astype(np.int64)
        ef = np.zeros((EP, EF), np.float32)
        ef[:e] = edge_feat[lo:hi]

        edge_in_packed = np.zeros((32, EP), np.float32)
        edge_in_packed[:EF, :] = ef.T
        edge_in_packed = edge_in_packed.astype(BF16)

        # gather rows in the AllGather table (NLP rows per rank)
        src_row = (srcr // NL) * NLP + srcr % NL
        src_row = np.concatenate([src_row, np.zeros(EP - e, np.int64)])
        dst_row = r * NLP + dstl
        dst_row = np.concatenate([dst_row, np.zeros(EP - e, np.int64)])

        def wrap_all(rows):
            return np.concatenate(
                [_wrap_idx(rows[g * GCH:(g + 1) * GCH]) for g in range(EP // GCH)],
                axis=1)

        # recip broadcast [128, NLP] f32
        counts = np.ones(NLP, np.float32)
        counts[:NL] = counts_full[r * NL:(r + 1) * NL]
        recip = (1.0 / counts).astype(np.float32)
        recip_bcast = np.tile(recip[None, :], (128, 1))

        # node input x unpacked [32, NLP]
        x_packed = np.zeros((32, NLP), np.float32)
        x_packed[:NF, :NL] = x[r * NL:(r + 1) * NL].T
        x_packed = x_packed.astype(BF16)

        cores.append(dict(
            E=e, dstl=dstl, edge_in_packed=edge_in_packed,
            idx_src=wrap_all(src_row), idx_dst=wrap_all(dst_row),
            recip_bcast=recip_bcast, x_packed=x_packed,
        ))
    return EP, cores


def _agg_schedule(EP, cores):
    """Canonical (uniform across cores) chunk windows per 128-node tile."""
    nch = EP // 128
    agg_win = []
    for t in range(NCH):
        c0, c1 = nch, 0
        for core in cores:
            dstl = core["dstl"]
            sel = np.nonzero((dstl >= 128 * t) & (dstl < 128 * (t + 1)))[0]
            if len(sel):
                c0 = min(c0, int(sel.min() // 128))
                c1 = max(c1, int(sel.max() // 128) + 1)
        if c0 >= c1:
            c0, c1 = 0, 1
        agg_win.append((c0, c1))
    return agg_win


def _build_S(EP, cores, agg_win):
    for core in cores:
        dstl = core["dstl"]
        e = len(dstl)
        blocks = []
        for t in range(NCH):
            c0, c1 = agg_win[t]
            for c in range(c0, c1):
                m = np.zeros((128, 128), np.float32)
                seg = dstl[c * 128:min((c + 1) * 128, e)]
                if len(seg):
                    sel = (seg >= t * 128) & (seg < (t + 1) * 128)
                    rows = np.nonzero(sel)[0]
                    m[rows, seg[sel] - t * 128] = 1.0
                blocks.append(m.astype(BF16))
        core["S_data"] = np.concatenate(blocks, axis=1)


def _prep_weights(inputs):
    f32 = np.float32
    W_ih_n = np.asarray(inputs["W_ih_n"], f32)
    W_hh_n = np.asarray(inputs["W_hh_n"], f32)
    b_n = np.asarray(inputs["b_ih_n"], f32) + np.asarray(inputs["b_hh_n"], f32)
    W_ih_e = np.asarray(inputs["W_ih_e"], f32)
    W_hh_e = np.asarray(inputs["W_hh_e"], f32)
    b_e = np.asarray(inputs["b_ih_e"], f32) + np.asarray(inputs["b_hh_e"], f32)
    W_node = np.asarray(inputs["W_node_mpn"], f32)   # [16, 256]
    b_node = np.asarray(inputs["b_node_mpn"], f32)
    W_edge = np.asarray(inputs["W_edge_mpn"], f32)   # [20, 384]
    b_edge = np.asarray(inputs["b_edge_mpn"], f32)
    W_pred = np.asarray(inputs["W_pred"], f32)       # [20, 4, 128]
    b_pred = np.asarray(inputs["b_pred"], f32)       # [20, 4]

    def hhT(W):  # [512, 128] -> [128, 512] col block g = gate g lhsT
        out = np.zeros((128, 512), f32)
        for g in range(4):
            out[:, 128 * g:128 * (g + 1)] = W[128 * g:128 * (g + 1), :].T
        return out.astype(BF16)

    def ihP(W, nfeat):  # x-part lhsT [32, 512]; rows 0..nfeat
        out = np.zeros((32, 512), f32)
        for g in range(4):
            out[:nfeat, 128 * g:128 * (g + 1)] = W[128 * g:128 * (g + 1), :].T
        return out.astype(BF16)

    def bias_cols(b):  # [512] -> [128, 4]
        return np.stack([b[128 * g:128 * (g + 1)] for g in range(4)], axis=1).astype(f32)

    def strip_bias(b, nfeat):  # [nfeat] -> [32, 1]
        out = np.zeros((32, 1), f32)
        out[:nfeat, 0] = b
        return out

    w = dict(
        WhhnT=hhT(W_hh_n), WihnP=ihP(W_ih_n, NF), bias_n=bias_cols(b_n),
        WhheT=hhT(W_hh_e), WiheP=ihP(W_ih_e, EF), bias_e=bias_cols(b_e),
        Wm1T=W_edge[:, 0:128].T.astype(BF16).copy(),
        Wm2T=W_edge[:, 128:256].T.astype(BF16).copy(),
        Wm3T=W_edge[:, 256:384].T.astype(BF16).copy(),
        bias_m=strip_bias(b_edge, EF),
        Wn1T=W_node[:, 0:128].T.astype(BF16).copy(),
        Wn2T=W_node[:, 128:256].T.astype(BF16).copy(),
        bias_nm=strip_bias(b_node, NF),
        WpT=np.concatenate([W_pred[i].T for i in range(NUM_ITER)], axis=1).astype(BF16),
        ident=np.eye(128, dtype=BF16),
        b_pred_sum=b_pred.sum(axis=0),  # added on host at the end
    )
    return w


def _build(EP, agg_win):
    nc = bacc.Bacc(num_devices=NCORES)
    dt = mybir.dt
    f32, bf16, i16 = dt.float32, dt.bfloat16, dt.int16

    ein = {}
    def EIn(name, shape, dty):
        ein[name] = nc.dram_tensor(name, shape, dty, kind="ExternalInput")
        return ein[name]

    n_aggblk = sum(c1 - c0 for c0, c1 in agg_win)

    x_edge = EIn("edge_in_packed", [32, EP], bf16)
    x_node = EIn("x_packed", [32, NLP], bf16)
    S_in = EIn("S_data", [128, n_aggblk * 128], bf16)
    idxs_in = EIn("idx_src", [128, EP // 16], i16)
    idxd_in = EIn("idx_dst", [128, EP // 16], i16)
    recip_in = EIn("recip_bcast", [128, NLP], f32)
    WhhnT_i = EIn("WhhnT", [128, 512], bf16)
    WihnP_i = EIn("WihnP", [32, 512], bf16)
    bias_n_i = EIn("bias_n", [128, 4], f32)
    WhheT_i = EIn("WhheT", [128, 512], bf16)
    WiheP_i = EIn("WiheP", [32, 512], bf16)
    bias_e_i = EIn("bias_e", [128, 4], f32)
    Wm1T_i = EIn("Wm1T", [128, EF], bf16)
    Wm2T_i = EIn("Wm2T", [128, EF], bf16)
    Wm3T_i = EIn("Wm3T", [128, EF], bf16)
    bias_m_i = EIn("bias_m", [32, 1], f32)
    Wn1T_i = EIn("Wn1T", [128, NF], bf16)
    Wn2T_i = EIn("Wn2T", [128, NF], bf16)
    bias_nm_i = EIn("bias_nm", [32, 1], f32)
    WpT_i = EIn("WpT", [128, 4 * NUM_ITER], bf16)
    ident_i = EIn("ident", [128, 128], bf16)

    out_ext = nc.dram_tensor("out", [NCLS, NLP], f32, kind="ExternalOutput")

    bounce = nc.dram_tensor("bounce_hn", [NCH, 128, 128], bf16, kind="Internal")
    table = nc.dram_tensor("table_hn", [NCORES * NLP, 128], bf16,
                           kind="Internal", addr_space="Shared")

    NSUP = EP // SUP          # edge gate supertiles
    NGR = EP // 2048          # msg groups (4 token tiles each)

    with TileContext(nc) as tc:
        with (
            tc.tile_pool(name="persist", bufs=1) as pp,
            tc.tile_pool(name="work", bufs=2) as wp,
            tc.tile_pool(name="work1", bufs=1) as wp1,
            tc.tile_pool(name="psA", bufs=2, space="PSUM") as psA,      # [128,1024] f32 gates
            tc.tile_pool(name="psB", bufs=2, space="PSUM") as psB,      # [128,512] misc
        ):
            # ---- persistent SBUF state ----
            h_eT = pp.tile([128, EP], bf16, tag="h_eT")
            c_e = pp.tile([128, EP], f32, tag="c_e")
            edge_in = pp.tile([32, EP], bf16, tag="edge_in")
            g_src = pp.tile([128, 2, GCH], bf16, tag="g_src")
            g_dst = pp.tile([128, 2, GCH], bf16, tag="g_dst")
            h_nT = pp.tile([128, NLP], bf16, tag="h_nT")
            c_n = pp.tile([128, NLP], f32, tag="c_n")
            h_nm = pp.tile([128, NCH, 128], bf16, tag="h_nm")   # node-major
            node_in = pp.tile([32, NLP], bf16, tag="node_in")
            agg = pp.tile([128, NLP], bf16, tag="agg")
            recip = pp.tile([128, NLP], f32, tag="recip")
            acc = pp.tile([NCLS, NLP], f32, tag="acc")
            S_d = pp.tile([128, n_aggblk * 128], bf16, tag="S_d")
            idxs = pp.tile([128, EP // 16], i16, tag="idxs")
            idxd = pp.tile([128, EP // 16], i16, tag="idxd")
            WhhnT = pp.tile([128, 512], bf16, tag="WhhnT")
            WihnP = pp.tile([32, 512], bf16, tag="WihnP")
            bias_n = pp.tile([128, 4], f32, tag="bias_n")
            WhheT = pp.tile([128, 512], bf16, tag="WhheT")
            WiheP = pp.tile([32, 512], bf16, tag="WiheP")
            bias_e = pp.tile([128, 4], f32, tag="bias_e")
            Wm1T = pp.tile([128, EF], bf16, tag="Wm1T")
            Wm2T = pp.tile([128, EF], bf16, tag="Wm2T")
            Wm3T = pp.tile([128, EF], bf16, tag="Wm3T")
            bias_m = pp.tile([32, 1], f32, tag="bias_m")
            Wn1T = pp.tile([128, NF], bf16, tag="Wn1T")
            Wn2T = pp.tile([128, NF], bf16, tag="Wn2T")
            bias_nm = pp.tile([32, 1], f32, tag="bias_nm")
            WpT = pp.tile([128, 4 * NUM_ITER], bf16, tag="WpT")
            ident = pp.tile([128, 128], bf16, tag="ident")

            # ---- load constants ----
            for tile_, src_ in [
                (S_d, S_in), (idxs, idxs_in), (idxd, idxd_in), (recip, recip_in),
                (WhhnT, WhhnT_i), (WihnP, WihnP_i), (bias_n, bias_n_i),
                (WhheT, WhheT_i), (WiheP, WiheP_i), (bias_e, bias_e_i),
                (Wm1T, Wm1T_i), (Wm2T, Wm2T_i), (Wm3T, Wm3T_i), (bias_m, bias_m_i),
                (Wn1T, Wn1T_i), (Wn2T, Wn2T_i), (bias_nm, bias_nm_i),
                (WpT, WpT_i), (ident, ident_i),
            ]:
                nc.sync.dma_start(tile_[:], src_[:])
            nc.sync.dma_start(edge_in[:], x_edge[:])
            nc.sync.dma_start(node_in[:], x_node[:])

            # ---- zero state ----
            nc.vector.memset(h_eT[:], 0.0)
            nc.vector.memset(c_e[:], 0.0)
            nc.vector.memset(h_nT[:], 0.0)
            nc.vector.memset(c_n[:], 0.0)
            nc.vector.memset(acc[:], 0.0)

            GATE_F = [AF.Sigmoid, AF.Sigmoid, AF.Tanh, AF.Sigmoid]  # i, f, g, o

            for it in range(NUM_ITER):
                # ============ node LSTM (two 640-token halves) ============
                if "node" in SKIP:
                    continue
                gates_n = []
                ntok = NLP // 4  # 320, packed token-tile width
                for g in range(4):
                    gs = wp.tile([128, NLP], f32, tag=f"gate{g}")
                    for half in range(2):
                        o0 = 640 * half
                        ps = psA.tile([128, SUP], f32, tag="gates_ps")
                        for (po, pw) in [(0, 512), (512, 128)]:
                            nc.tensor.matmul(
                                ps[:, po:po + pw],
                                WhhnT[:, 128 * g:128 * (g + 1)],
                                h_nT[:, o0 + po:o0 + po + pw], start=True,
                                stop=False, skip_group_check=True)
                        for fi, (po, pw) in enumerate([(0, 512), (512, 128)]):
                            nc.tensor.matmul(
                                ps[:, po:po + pw],
                                WihnP[:, 128 * g:128 * (g + 1)],
                                node_in[:, o0 + po:o0 + po + pw],
                                start=False, stop=(fi == 1),
                                skip_group_check=True)
                        nc.scalar.activation(gs[:, o0:o0 + 640], ps[:, :640],
                                             GATE_F[g], bias=bias_n[:, g:g + 1])
                    gates_n.append(gs)
                i_g, f_g, g_g, o_g = gates_n
                t2 = wp1.tile([128, NLP], f32, tag="t2")
                nc.vector.tensor_mul(c_n[:], c_n[:], f_g[:])
                nc.vector.tensor_mul(t2[:], i_g[:], g_g[:])
                nc.vector.tensor_add(c_n[:], c_n[:], t2[:])
                th = wp1.tile([128, NLP], f32, tag="th")
                nc.scalar.activation(th[:], c_n[:], AF.Tanh)
                nc.vector.tensor_mul(h_nT[:], o_g[:], th[:])

                # prediction head: acc += WpT[it].T @ h_nT
                for o in range(0, NLP, 512) if "pred" not in SKIP else []:
                    w = min(512, NLP - o)
                    ps = psB.tile([128, 512], f32, tag="ps_small")
                    nc.tensor.matmul(ps[:NCLS, :w], WpT[:, 4 * it:4 * (it + 1)],
                                     h_nT[:, o:o + w])
                    nc.vector.tensor_add(acc[:, o:o + w], acc[:, o:o + w],
                                         ps[:NCLS, :w])

                # node-major h_n + DRAM table + AllGather
                for cb in (range(0, NCH, 4) if "tr" not in SKIP else []):
                    nb = min(4, NCH - cb)
                    pst = psB.tile([128, 512], bf16, tag="ps_small")
                    for c in range(nb):
                        nc.tensor.transpose(
                            pst[:, 128 * c:128 * (c + 1)],
                            h_nT[:, 128 * (cb + c):128 * (cb + c + 1)], ident[:])
                    nc.vector.tensor_copy(
                        h_nm[:, cb:cb + nb, :].rearrange("p a b -> p (a b)"),
                        pst[:, :128 * nb])
                if "cc" not in SKIP:
                  nc.sync.dma_start(bounce[:].rearrange("c p f -> p c f"), h_nm[:])
                  nc.gpsimd.collective_compute(
                    "AllGather", mybir.AluOpType.bypass,
                    replica_groups=[list(range(NCORES))],
                    ins=[bounce[:].opt()], outs=[table[:].opt()],
                )

                # ============ edge LSTM (supertiles of SUP tokens) ============
                for s in (range(NSUP) if "edge" not in SKIP else []):
                    base = s * SUP
                    gates_e = []
                    for g in range(4):
                        ps = psA.tile([128, SUP], f32, tag="gates_ps")
                        for q in range(SUP // 512):
                            o = base + 512 * q
                            nc.tensor.matmul(
                                ps[:, 512 * q:512 * (q + 1)],
                                WhheT[:, 128 * g:128 * (g + 1)],
                                h_eT[:, o:o + 512], start=True, stop=False,
                                skip_group_check=True)
                        for q in range(SUP // 512):
                            t = (base + 512 * q) // 512
                            nc.tensor.matmul(
                                ps[:, 512 * q:512 * (q + 1)],
                                WiheP[:, 128 * g:128 * (g + 1)],
                                edge_in[:, 512 * t:512 * (t + 1)],
                                start=False, stop=True, skip_group_check=True)
                        gs_full = wp.tile([128, NLP], f32, tag=f"gate{g}")
                        gs = gs_full[:, :SUP]
                        nc.scalar.activation(gs[:], ps[:], GATE_F[g],
                                             bias=bias_e[:, g:g + 1])
                        gates_e.append(gs)
                    i_g, f_g, g_g, o_g = gates_e
                    sl = slice(base, base + SUP)
                    t2_full = wp1.tile([128, NLP], f32, tag="t2")
                    t2 = t2_full[:, :SUP]
                    nc.vector.tensor_mul(c_e[:, sl], c_e[:, sl], f_g[:])
                    nc.vector.tensor_mul(t2[:], i_g[:], g_g[:])
                    nc.vector.tensor_add(c_e[:, sl], c_e[:, sl], t2[:])
                    th_full = wp1.tile([128, NLP], f32, tag="th")
                    th = th_full[:, :SUP]
                    nc.scalar.activation(th[:], c_e[:, sl], AF.Tanh)
                    nc.vector.tensor_mul(h_eT[:, sl], o_g[:], th[:])

                # ============ aggregation (scatter-mean via S matmuls) ======
                blk = 0
                for t in (range(NCH) if "agg" not in SKIP else []):
                    c0, c1 = agg_win[t]
                    psag = psB.tile([128, 128], f32, tag="ps_small")
                    for ci, c in enumerate(range(c0, c1)):
                        # transpose h_e chunk c to edge-major
                        pst = psB.tile([128, 128], bf16, tag="ps_tr")
                        nc.tensor.transpose(pst[:], h_eT[:, 128 * c:128 * (c + 1)],
                                            ident[:])
                        trb = wp.tile([128, 128], bf16, tag="he_em")
                        nc.vector.tensor_copy(trb[:], pst[:])
                        nc.tensor.matmul(
                            psag[:], trb[:], S_d[:, 128 * blk:128 * (blk + 1)],
                            start=(ci == 0), stop=(ci == c1 - c0 - 1),
                            skip_group_check=True)
                        blk += 1
                    nc.vector.tensor_mul(agg[:, 128 * t:128 * (t + 1)], psag[:],
                                         recip[:, 128 * t:128 * (t + 1)])

                # ===== edge messages (gathers interleaved, 2-slot rotation) =====
                IW = GCH // 16  # idx columns per gather chunk
                for grp in (range(NGR) if "msg" not in SKIP else []):
                    slot = grp % 2
                    if "gather" in SKIP:
                        pass
                    else:
                        nc.gpsimd.dma_gather(
                            g_src[:, slot:slot + 1, :], table[:],
                            idxs[:, IW * grp:IW * (grp + 1)],
                            num_idxs=GCH, num_idxs_reg=GCH, elem_size=H,
                            transpose=True, single_packet=False)
                        nc.gpsimd.dma_gather(
                            g_dst[:, slot:slot + 1, :], table[:],
                            idxd[:, IW * grp:IW * (grp + 1)],
                            num_idxs=GCH, num_idxs_reg=GCH, elem_size=H,
                            transpose=True, single_packet=False)
                    for j in range(4):
                        t = 4 * grp + j
                        psm = psB.tile([128, 512], f32, tag="ps_small")
                        nc.tensor.matmul(psm[:EF, :], Wm1T[:],
                                         g_src[:, slot, 512 * j:512 * (j + 1)],
                                         start=True, stop=False,
                                         skip_group_check=True)
                        nc.tensor.matmul(psm[:EF, :], Wm3T[:],
                                         g_dst[:, slot, 512 * j:512 * (j + 1)],
                                         start=False, stop=False,
                                         skip_group_check=True)
                        nc.tensor.matmul(psm[:EF, :], Wm2T[:],
                                         h_eT[:, 512 * t:512 * (t + 1)],
                                         start=False, stop=True,
                                         skip_group_check=True)
                        nc.scalar.activation(
                            edge_in[:EF, 512 * t:512 * (t + 1)],
                            psm[:EF, :], AF.Lrelu,
                            bias=bias_m[:EF, :], alpha=NEG)

                # ============ node messages ============
                ntok = NLP // 4
                for j in (range(4) if "nmsg" not in SKIP else []):
                    psn = psB.tile([128, 512], f32, tag="ps_small")
                    nc.tensor.matmul(psn[:NF, :ntok], Wn1T[:],
                                     agg[:, ntok * j:ntok * (j + 1)],
                                     start=True, stop=False, skip_group_check=True)
                    nc.tensor.matmul(psn[:NF, :ntok], Wn2T[:],
                                     h_nT[:, ntok * j:ntok * (j + 1)],
                                     start=False, stop=True, skip_group_check=True)
                    nc.scalar.activation(
                        node_in[:NF, ntok * j:ntok * (j + 1)],
                        psn[:NF, :ntok], AF.Lrelu,
                        bias=bias_nm[:NF, :], alpha=NEG)

            # ---- output ----
            nc.sync.dma_start(out_ext[:], acc[:])

    nc.finalize()
    return nc


_BUILD_CACHE = {}


def kernel(**inputs):
    global LAST_RESULT
    EP, cores = _preprocess(inputs)
    agg_win = _agg_schedule(EP, cores)
    _build_S(EP, cores, agg_win)
    w = _prep_weights(inputs)

    key = (EP, tuple(agg_win))
    if key not in _BUILD_CACHE:
        _BUILD_CACHE[key] = _build(EP, agg_win)
    nc = _BUILD_CACHE[key]

    in_maps = []
    for r in range(NCORES):
        c = cores[r]
        in_maps.append({
            "edge_in_packed": np.ascontiguousarray(c["edge_in_packed"]),
            "x_packed": np.ascontiguousarray(c["x_packed"]),
            "S_data": np.ascontiguousarray(c["S_data"]),
            "idx_src": np.ascontiguousarray(c["idx_src"]),
            "idx_dst": np.ascontiguousarray(c["idx_dst"]),
            "recip_bcast": np.ascontiguousarray(c["recip_bcast"]),
            "WhhnT": w["WhhnT"], "WihnP": w["WihnP"], "bias_n": w["bias_n"],
            "WhheT": w["WhheT"], "WiheP": w["WiheP"], "bias_e": w["bias_e"],
            "Wm1T": w["Wm1T"], "Wm2T": w["Wm2T"], "Wm3T": w["Wm3T"],
            "bias_m": w["bias_m"],
            "Wn1T": w["Wn1T"], "Wn2T": w["Wn2T"], "bias_nm": w["bias_nm"],
            "WpT": w["WpT"], "ident": w["ident"],
        })

    import time as _time
    t0 = _time.monotonic()
    res = run_bass_kernel_spmd(nc, in_maps, core_ids=list(range(NCORES)),
                               trace=False)
    res.wall_ns = int((_time.monotonic() - t0) * 1e9)
    LAST_RESULT = res

    out = np.zeros((N, NCLS), np.float32)
    for r in range(NCORES):
        o = res.results[r]["out"]  # [4, NLP]
        out[r * NL:(r + 1) * NL] = o[:, :NL].T
    out += w["b_pred_sum"][None, :]
    return out
